# revision 17
# baseline (speedup 1.0000x reference)
"""Trainium2 Bass kernel for nn_DecoderRNN (Bahdanau-attention GRU decoder).

v4: attention tanh (ScalarE) is the per-step floor; everything else hides
under it.  Key changes vs v3:
- cnn_proj stored fp8 RESIDENT in SBUF ((b,n) column order) -> no per-step
  proj streaming; broadcast hq-add reads fp8 directly (DVE/GpSimd split).
- all PE transposes replaced by DMA-transpose XBAR ([16,X] -> [128,(k,b)]).
- scores psum chunks placed at col-groups 0/32/64/96 -> 2 banks, freeing
  6 banks for k-outer gh accumulation in the window.
- col-tiled (tile_position) matmuls for the thin M=16 GEMMs (hq/ctx/gi).
- whhT (window) and wxT (tail) streamed from HBM under the tanh shadow;
  featp + whT resident.
- scores evacuated via (b,n)-ordered psum -> one reshape DMA to [16,196].
"""
import os
import sys

sys.path.insert(0, "/opt/trn_rl_repo")

import numpy as np
import ml_dtypes

import concourse.bass as bass
import concourse.tile as tile
from concourse import mybir
from concourse.bass_utils import run_bass_kernel_spmd
from concourse.masks import make_identity

F32 = mybir.dt.float32
BF16 = mybir.dt.bfloat16
FP8 = mybir.dt.float8e4
bf = ml_dtypes.bfloat16
f8 = ml_dtypes.float8_e4m3
AL = mybir.AluOpType
AF = mybir.ActivationFunctionType

NCORES = 8
B = 16            # local batch per core
N = 196           # attention positions
H = 1024          # hidden
E = 512           # embed dim
G = 3 * H         # gate width
T = int(os.environ.get("DECODER_STEPS", "17"))
C = 1000          # classes
NB = N * B        # 3136 ((b,n) cols, b-major)
KH = 8            # h k-tiles (1024/128)
KB = 32           # padded (b,n) k-tiles for ctx (16*256/128)
SCW = 448         # scores chunk width
NSC = 7           # scores chunks (7*448 = 3136)
CT = 8            # classifier m-tiles (1024 padded)
TB = T * B

_CACHE = {}


def _split_waits(nc, keep=1):
    """This container's walrus build rejects >1 sem-wait per instruction
    (setupSyncWait: 'Too many sync wait commands'). Hoist all but one wait
    of every instruction onto single-wait NoOps on the same engine, placed
    immediately before it in program order."""
    nfix = 0
    for bb in nc.main_func.blocks:
        il = bb.instructions
        i = 0
        while i < len(il):
            ins = il[i]
            si = getattr(ins, 'sync_info', None)
            if si is not None and len(si.on_wait) > keep:
                waits = list(si.on_wait)
                for w_i, w in enumerate(waits[:-keep]):
                    nop = mybir.InstNoOp(name=f"{ins.name}-ws{w_i}", ins=[],
                                         outs=[])
                    nop.engine = ins.engine
                    nop.sync_info = mybir.SyncInfo(on_wait=[w], on_update=[])
                    il.insert(i, nop)
                    i += 1
                ins.sync_info = mybir.SyncInfo(on_wait=waits[-keep:],
                                               on_update=list(si.on_update))
                nfix += 1
            i += 1
    return nfix


def _build_program():
    nc = bass.Bass()

    featTc_d = nc.declare_dram_parameter("featTc", [NSC, 128, KH, SCW], BF16,
                                         isOutput=False)
    featp_d = nc.declare_dram_parameter("featp", [KB, 128, H], BF16, isOutput=False)
    wcT_d = nc.declare_dram_parameter("wcT", [KH, 128, H], BF16, isOutput=False)
    bcpk_d = nc.declare_dram_parameter("bcpk", [128, KH], F32, isOutput=False)
    wxT_d = nc.declare_dram_parameter("wxT", [KH, 128, G], BF16, isOutput=False)
    whhT_d = nc.declare_dram_parameter("whhT", [KH, 128, G], BF16, isOutput=False)
    whT_d = nc.declare_dram_parameter("whT", [KH, 128, H], BF16, isOutput=False)
    wclsT_d = nc.declare_dram_parameter("wclsT", [KH, 128, CT * 128], BF16,
                                        isOutput=False)
    vT_d = nc.declare_dram_parameter("vT", [128, KH], BF16, isOutput=False)
    ge_d = nc.declare_dram_parameter("ge", [T, B, G], BF16, isOutput=False)
    h0b_d = nc.declare_dram_parameter("h0b", [B, H], F32, isOutput=False)
    hpk0_d = nc.declare_dram_parameter("hpk0", [128, 128], BF16, isOutput=False)
    out_d = nc.declare_dram_parameter("out", [CT, 128, TB], F32, isOutput=True)

    with tile.TileContext(nc) as tc:
        with tc.tile_pool(name="persist", bufs=1) as P1, \
             tc.tile_pool(name="state", bufs=2) as P2:

            # ---- persistent tensors
            proj8 = P1.tile([128, KH, NB], BF16)     # resident cnn_proj (+bc+bh)
            feat_s = P1.tile([128, KB, H], BF16)
            hs_sb = P1.tile([128, KH, T, B], BF16)   # h history for classifier
            vT_s = P1.tile([128, KH], BF16)
            ident16 = P1.tile([B, B], BF16)
            bc_pk = P1.tile([128, KH], F32)
            wblk = P1.tile([128, 33 * B], BF16)
            w16p = P1.tile([B, 256], BF16)           # normalized softmax w, padded
            h32 = P1.tile([B, H], F32)

            nc.sync.dma_start(vT_s, vT_d[:])
            make_identity(nc, ident16)
            nc.sync.dma_start(bc_pk, bcpk_d[:])
            nc.sync.dma_start(h32, h0b_d[:])
            nc.vector.memset(wblk, 0.0)
            nc.vector.memset(w16p, 0.0)
            for kb in range(KB):
                nc.gpsimd.dma_start(feat_s[:, kb, :], featp_d[kb])

            hpk = P2.tile([128, 128], BF16, tag="hpk")
            nc.sync.dma_start(hpk, hpk0_d[:])

            # ---- startup: proj8 = fp8(feat @ Wc^T + bc), (b,n) columns
            with tc.tile_pool(name="wcpool", bufs=1) as Pwc, \
                 tc.tile_pool(name="ftring", bufs=2) as Pft, \
                 tc.tile_pool(name="ps_start", bufs=3, space="PSUM") as PSs:
                wcT_s = Pwc.tile([128, KH, H], BF16)
                for k in range(KH):
                    nc.sync.dma_start(wcT_s[:, k, :], wcT_d[k])
                for cch in range(NSC):
                    ft = Pft.tile([128, KH, SCW], BF16, tag="ft",
                                  name=f"ft{cch}")
                    for q in range(2):
                        ql = slice(q * (KH // 2), (q + 1) * (KH // 2))
                        nc.gpsimd.dma_start(ft[:, ql, :], featTc_d[cch][:, ql, :])
                    for m in range(KH):
                        ps = PSs.tile([128, SCW], F32, tag="ps",
                                      name=f"ps{cch}_{m}")
                        for k in range(KH):
                            nc.tensor.matmul(
                                ps, wcT_s[:, k, m * 128:(m + 1) * 128],
                                ft[:, k, :],
                                start=(k == 0), stop=(k == KH - 1))
                        dst = proj8[:, m, cch * SCW:(cch + 1) * SCW]
                        if m % 2 == 0:
                            nc.vector.tensor_scalar(
                                out=dst, in0=ps, scalar1=bc_pk[:, m:m + 1],
                                scalar2=None, op0=AL.add)
                        else:
                            nc.scalar.activation(
                                dst, ps, AF.Identity,
                                bias=bc_pk[:, m:m + 1])

            # ---- time loop
            with tc.tile_pool(name="whhring", bufs=2) as Pwhh, \
                 tc.tile_pool(name="whtring", bufs=2) as Pwht, \
                 tc.tile_pool(name="wxring", bufs=2) as Pwx, \
                 tc.tile_pool(name="gering", bufs=1) as Pge, \
                 tc.tile_pool(name="xring", bufs=2) as Px, \
                 tc.tile_pool(name="small", bufs=1) as Psm, \
                 tc.tile_pool(name="gt", bufs=2) as Pgt, \
                 tc.tile_pool(name="gf", bufs=1) as Pgf:
                for t in range(T):
                    ge_t = Pge.tile([B, G], BF16, tag="ge", name=f"ge{t}")
                    nc.gpsimd.dma_start(ge_t, ge_d[t])

                    def dummy(pool, anchor, i):
                        """Anchored keep-warm matmul: fires right after the
                        glue op that writes `anchor`, resetting the HAM idle
                        clock so the PE stays at K=8/8."""
                        dps = pool.tile([B, 64], F32, tag="dum",
                                        name=f"dum{t}_{i}")
                        nc.tensor.matmul(dps, ident16, anchor,
                                         start=True, stop=True)

                    # ---- hq = h @ Wh^T (ct4 over k), packed via DMA-transpose
                    hqf = Psm.tile([B, H], BF16, tag="hqf", name=f"hqf{t}")
                    with tc.tile_pool(name="psA", bufs=2, space="PSUM") as PA, \
                         tc.tile_pool(name="psAd", bufs=2, space="PSUM") as PAd:
                        hq_pss = []
                        for c in range(2):
                            hq_pss.append(PA.tile([128, 512], F32, tag="hqp",
                                                  name=f"hqp{t}_{c}"))
                        for k in range(KH):
                            wht_t = Pwht.tile([128, H], BF16, tag="wht",
                                              name=f"wht{t}_{k}")
                            nc.sync.dma_start(wht_t, whT_d[k])
                            g = k % 4
                            for c in range(2):
                                nc.tensor.matmul(
                                    hq_pss[c][32 * g:32 * g + B, :],
                                    hpk[:, k * B:(k + 1) * B],
                                    wht_t[:, c * 512:(c + 1) * 512],
                                    start=(k < 4), stop=(k >= 4),
                                    tile_position=(0, 32 * g))
                        for c in range(2):
                            ps = hq_pss[c]
                            if True:
                                pass
                            t1 = Psm.tile([B, 512], F32, tag="red",
                                          name=f"hqt1_{t}_{c}", bufs=2)
                            nc.vector.tensor_copy(t1, ps[32:32 + B, :])
                            t2 = Psm.tile([B, 512], F32, tag="red",
                                          name=f"hqt2_{t}_{c}", bufs=2)
                            nc.vector.tensor_tensor(
                                out=t2, in0=ps[64:64 + B, :], in1=t1, op=AL.add)
                            t3 = Psm.tile([B, 512], F32, tag="red",
                                          name=f"hqt3_{t}_{c}", bufs=2)
                            nc.vector.tensor_tensor(
                                out=t3, in0=ps[96:96 + B, :], in1=t2, op=AL.add)
                            nc.vector.tensor_tensor(
                                out=hqf[:, c * 512:(c + 1) * 512],
                                in0=ps[0:B, :], in1=t3, op=AL.add)
                        dummy(PAd, hqf[:, 0:64], "hqf")
                    hq_sb = Psm.tile([128, 128], BF16, tag="hqsb",
                                     name=f"hqsb{t}", bufs=2)
                    nc.sync.dma_start(
                        out=hq_sb.rearrange("p (k b) -> p k b", b=B),
                        in_=hqf, transpose=True)

                    # ---- joint gh+gi psum: ge injected via identity matmul,
                    # gh accumulates in the window, gi joins in the tail.
                    hn_sb = Psm.tile([B, H], BF16, tag="hn", name=f"hn{t}")
                    scores_sb = Psm.tile([B, N], BF16, tag="scores",
                                         name=f"scores{t}")
                    srz = Psm.tile([B, 2 * H], BF16, tag="srz", name=f"srz{t}")
                    xbf_last = None
                    with tc.tile_pool(name="psJ", bufs=1, space="PSUM") as PJ:
                        rz_ps = [PJ.tile([B, 512], F32, tag=f"rz{c}",
                                         name=f"rz{t}_{c}") for c in range(4)]
                        hn_ps = [PJ.tile([B, 512], F32, tag=f"hnp{c}",
                                         name=f"hnp{t}_{c}") for c in range(2)]
                        for c in range(4):
                            nc.tensor.matmul(
                                rz_ps[c], ident16,
                                ge_t[:, c * 512:(c + 1) * 512],
                                start=True, stop=False)

                        with tc.tile_pool(name="psB", bufs=1,
                                          space="PSUM") as PB:
                            sc_banks = [PB.tile([128, SCW], F32,
                                                tag=f"scb{i}",
                                                name=f"scb{t}_{i}")
                                        for i in range(2)]

                            def sc_slice(c):
                                g = c // 2
                                return sc_banks[c % 2][32 * g:32 * g + 1, :]

                            for hi in range(KH):
                                # gh k-tile first: its inputs (hpk, whh) are
                                # ready, so the PE works through it while the
                                # ACT engine runs this tile's tanh.
                                whh_t = Pwhh.tile([128, G], BF16, tag="whh",
                                                  name=f"whh{t}_{hi}")
                                for qq in range(2):
                                    ql = slice(qq * (G // 2),
                                               (qq + 1) * (G // 2))
                                    nc.gpsimd.dma_start(whh_t[:, ql],
                                                        whhT_d[hi][:, ql])
                                for c in range(6):
                                    tgt = rz_ps[c] if c < 4 else hn_ps[c - 4]
                                    nc.tensor.matmul(
                                        tgt,
                                        hpk[:, hi * B:(hi + 1) * B],
                                        whh_t[:, c * 512:(c + 1) * 512],
                                        start=(c >= 4 and hi == 0),
                                        stop=(c >= 4 and hi == KH - 1))
                                xbf = Px.tile([128, NB], BF16, tag="x",
                                              name=f"x{t}_{hi}")
                                if hi == KH - 1:
                                    xbf_last = xbf
                                x3 = xbf.rearrange("p (b n) -> p b n", n=N)
                                p3 = proj8[:, hi, :].rearrange(
                                    "p (b n) -> p b n", n=N)
                                hqb = hq_sb[:, hi * B:(hi + 1) * B] \
                                    .unsqueeze(2).broadcast_to([128, B, N])
                                nc.vector.tensor_tensor(out=x3, in0=p3,
                                                        in1=hqb, op=AL.add)
                                nc.scalar.activation(xbf, xbf, AF.Tanh)
                                for c in range(NSC):
                                    nc.tensor.matmul(
                                        sc_slice(c),
                                        vT_s[:, hi:hi + 1],
                                        xbf[:, c * SCW:(c + 1) * SCW],
                                        start=(hi == 0), stop=(hi == KH - 1),
                                        tile_position=(0, 32 * (c // 2)))
                            # scores evac into row 0 of last pre-tanh buffer
                            scflat = xbf_last[0:1, :]
                            for c in range(NSC):
                                seg = scflat[:, c * SCW:(c + 1) * SCW]
                                if c % 2 == 0:
                                    nc.vector.tensor_copy(seg, sc_slice(c))
                                else:
                                    nc.scalar.activation(seg, sc_slice(c),
                                                         AF.Copy)
                            nc.sync.dma_start(
                                out=scores_sb,
                                in_=scflat.rearrange("o (b n) -> o b n", n=N))
                        # hn evac (gh n-part complete at window end)
                        nc.vector.tensor_copy(hn_sb[:, 0:512], hn_ps[0])
                        nc.scalar.activation(hn_sb[:, 512:1024], hn_ps[1],
                                             AF.Copy)

                        # ---- softmax -> normalized weights -> wblk
                        exps = Psm.tile([B, N], BF16, tag="exps",
                                        name=f"exps{t}")
                        sumexp = Psm.tile([B, 1], F32, tag="sumexp",
                                          name=f"sumexp{t}")
                        nc.scalar.activation(exps, scores_sb, AF.Exp,
                                             accum_out=sumexp)
                        rec = Psm.tile([B, 1], F32, tag="rec", name=f"rec{t}")
                        nc.vector.reciprocal(rec, sumexp)
                        nc.vector.tensor_scalar(out=w16p[:, 0:N], in0=exps,
                                                scalar1=rec, scalar2=None,
                                                op0=AL.mult)
                        wT_t = Psm.tile([128, 2, B], BF16, tag="wTt",
                                        name=f"wTt{t}", bufs=2)
                        nc.sync.dma_start(out=wT_t, in_=w16p, transpose=True)
                        wv = wblk.rearrange("p (b r) -> p b r", r=33)
                        nc.sync.dma_start(out=wv[:, :, 0:1],
                                          in_=wT_t[:, 0, :].unsqueeze(2))
                        nc.scalar.dma_start(out=wv[0:68, :, 16:17],
                                            in_=wT_t[0:68, 1, :].unsqueeze(2))

                        # ---- ctx (ct4 over kb) + keep-warm dummies
                        ctxs = Psm.tile([B, H], BF16, tag="ctxs",
                                        name=f"ctxs{t}")
                        def dummy_into(anchor):
                            # hn_ps[1] is dead after its evac; reuse its bank
                            nc.tensor.matmul(hn_ps[1][:, 0:64], ident16,
                                             anchor, start=True, stop=True)

                        with tc.tile_pool(name="psC", bufs=2,
                                          space="PSUM") as PC:
                            dummy_into(exps[:, 0:64])
                            dummy_into(w16p[:, 0:64])
                            for c in range(2):
                                ps = PC.tile([128, 512], F32, tag="ctxp",
                                             name=f"ctxp{t}_{c}")
                                for kb in range(KB):
                                    g = kb % 4
                                    nc.tensor.matmul(
                                        ps[32 * g:32 * g + B, :],
                                        wblk[:, kb * B:(kb + 1) * B],
                                        feat_s[:, kb, c * 512:(c + 1) * 512],
                                        start=(kb < 4), stop=(kb >= KB - 4),
                                        tile_position=(0, 32 * g))
                                t1 = Psm.tile([B, 512], F32, tag="red",
                                              name=f"ctx1_{t}_{c}", bufs=2)
                                nc.vector.tensor_copy(t1, ps[32:32 + B, :])
                                t2 = Psm.tile([B, 512], F32, tag="red",
                                              name=f"ctx2_{t}_{c}", bufs=2)
                                nc.vector.tensor_tensor(
                                    out=t2, in0=ps[64:64 + B, :], in1=t1,
                                    op=AL.add)
                                t3 = Psm.tile([B, 512], F32, tag="red",
                                              name=f"ctx3_{t}_{c}", bufs=2)
                                nc.vector.tensor_tensor(
                                    out=t3, in0=ps[96:96 + B, :], in1=t2,
                                    op=AL.add)
                                nc.vector.tensor_tensor(
                                    out=ctxs[:, c * 512:(c + 1) * 512],
                                    in0=ps[0:B, :], in1=t3, op=AL.add)
                        ctxT = Psm.tile([128, 128], BF16, tag="ctxT",
                                        name=f"ctxT{t}", bufs=2)
                        ctxT3 = ctxT.rearrange("p (k b) -> p k b", b=B)
                        nc.scalar.dma_start(out=ctxT3[:, 0:4, :],
                                            in_=ctxs[:, 0:512],
                                            transpose=True)
                        nc.scalar.dma_start(out=ctxT3[:, 4:8, :],
                                            in_=ctxs[:, 512:1024],
                                            transpose=True)

                        # ---- gi joins the joint psum (k-outer, wx streamed)
                        narg = Psm.tile([B, H], BF16, tag="narg",
                                        name=f"narg{t}")
                        with tc.tile_pool(name="psD", bufs=1,
                                          space="PSUM") as PD:
                            gin_ps = [PD.tile([B, 512], F32, tag=f"gin{c}",
                                              name=f"gin{t}_{c}")
                                      for c in range(2)]
                            for c in range(2):
                                nc.tensor.matmul(
                                    gin_ps[c], ident16,
                                    ge_t[:, 2 * H + c * 512:
                                         2 * H + (c + 1) * 512],
                                    start=True, stop=False)
                            dummy_into(ctxs[:, 0:64])
                            for k in range(KH):
                                wx_t = Pwx.tile([128, G], BF16, tag="wx",
                                                name=f"wx{t}_{k}")
                                for qq in range(2):
                                    ql = slice(qq * (G // 2),
                                               (qq + 1) * (G // 2))
                                    nc.gpsimd.dma_start(wx_t[:, ql],
                                                        wxT_d[k][:, ql])
                                for c in range(6):
                                    tgt = rz_ps[c] if c < 4 else gin_ps[c - 4]
                                    nc.tensor.matmul(
                                        tgt,
                                        ctxT[:, k * B:(k + 1) * B],
                                        wx_t[:, c * 512:(c + 1) * 512],
                                        start=False, stop=(k == KH - 1))
                            # srz = 0.5 * (gh + gi + ge_raw)
                            for c in range(4):
                                dst = srz[:, c * 512:(c + 1) * 512]
                                if c % 2 == 0:
                                    nc.vector.tensor_scalar(
                                        out=dst, in0=rz_ps[c], scalar1=0.5,
                                        scalar2=None, op0=AL.mult)
                                else:
                                    nc.scalar.mul(dst, rz_ps[c], 0.5)
                            # GRU gates start (narg reads gi-n psum directly)
                            nc.scalar.activation(srz[:, 0:H], srz[:, 0:H],
                                                 AF.Tanh)
                            nc.scalar.activation(srz[:, H:2 * H],
                                                 srz[:, H:2 * H], AF.Tanh)
                            r_ = Pgt.tile([B, H], BF16, tag="gt",
                                          name=f"r{t}")
                            nc.vector.tensor_scalar(out=r_, in0=srz[:, 0:H],
                                                    scalar1=0.5, scalar2=0.5,
                                                    op0=AL.mult, op1=AL.add)
                            rhn = Pgt.tile([B, H], BF16, tag="gt",
                                           name=f"rhn{t}")
                            nc.vector.tensor_tensor(out=rhn, in0=r_,
                                                    in1=hn_sb, op=AL.mult)
                            dummy_into(srz[:, 0:64])
                            for c in range(2):
                                nc.vector.tensor_tensor(
                                    out=narg[:, c * 512:(c + 1) * 512],
                                    in0=gin_ps[c],
                                    in1=rhn[:, c * 512:(c + 1) * 512],
                                    op=AL.add)
                    # ---- rest of GRU elementwise
                    with tc.tile_pool(name="psDum", bufs=2,
                                      space="PSUM") as PDm:
                        n_ = Pgf.tile([B, H], F32, tag="gf", name=f"n{t}")
                        nc.scalar.activation(n_, narg, AF.Tanh)
                        z_ = Pgt.tile([B, H], BF16, tag="gt", name=f"z{t}")
                        nc.gpsimd.tensor_scalar(out=z_, in0=srz[:, H:2 * H],
                                                scalar1=0.5, scalar2=0.5,
                                                op0=AL.mult, op1=AL.add)
                        dummy(PDm, narg[:, 0:64], "narg")
                        d_ = Pgf.tile([B, H], BF16, tag="gfd", name=f"d{t}")
                        nc.vector.tensor_tensor(out=d_, in0=h32, in1=n_,
                                                op=AL.subtract)
                        zd = Pgt.tile([B, H], BF16, tag="gt", name=f"zd{t}")
                        nc.vector.tensor_tensor(out=zd, in0=z_, in1=d_,
                                                op=AL.mult)
                        dummy(PDm, zd[:, 0:64], "zd")
                        nc.vector.tensor_tensor(out=h32, in0=n_, in1=zd,
                                                op=AL.add)
                        h16f = Pgt.tile([B, H], BF16, tag="gt",
                                        name=f"h16f{t}")
                        nc.vector.tensor_copy(h16f, h32)
                        dummy(PDm, h16f[:, 0:64], "h16f")
                        hpk_n = P2.tile([128, 128], BF16, tag="hpk",
                                        name=f"hpk{t}")
                        nc.scalar.dma_start(
                            out=hpk_n.rearrange("p (k b) -> p k b", b=B),
                            in_=h16f, transpose=True)
                        nc.vector.tensor_copy(
                            hs_sb[:, :, t, :],
                            hpk_n.rearrange("p (k b) -> p k b", b=B))
                        hpk = hpk_n

            # ---- classifier from SBUF h history
            with tc.tile_pool(name="clsw", bufs=1) as Pc, \
                 tc.tile_pool(name="outst", bufs=2) as Po, \
                 tc.tile_pool(name="psE", bufs=2, space="PSUM") as PEp:
                wcls_s = Pc.tile([128, KH, CT * 128], BF16)
                for k in range(KH):
                    for q in range(4):
                        ql = slice(q * CT * 32, (q + 1) * CT * 32)
                        nc.sync.dma_start(wcls_s[:, k, ql],
                                          wclsT_d[k][:, ql])
                for mc in range(CT):
                    ps = PEp.tile([128, TB], F32, tag="cls", name=f"cls{mc}")
                    for k in range(KH):
                        nc.tensor.matmul(
                            ps,
                            wcls_s[:, k, mc * 128:(mc + 1) * 128],
                            hs_sb[:, k, :, :],
                            start=(k == 0), stop=(k == KH - 1))
                    ot = Po.tile([128, TB], F32, tag="ot", name=f"ot{mc}")
                    if mc % 2 == 0:
                        nc.vector.tensor_copy(ot, ps)
                    else:
                        nc.scalar.activation(ot, ps, AF.Copy)
                    nc.sync.dma_start(out_d[mc], ot)

    _split_waits(nc)
    return nc


def _get_program():
    if "nc" not in _CACHE:
        _CACHE["nc"] = _build_program()
    return _CACHE["nc"]


def _pack_inputs(cnn_feat, labels, sos, h0, embed_table, W_ih, b_ih, W_hh,
                 b_hh, Wh, bh, Wc, bc, v_w, Wcls):
    """Host-side layout prep. Returns list of per-core input dicts."""
    f32 = np.float32
    cnn_feat = np.asarray(cnn_feat, f32)
    labels = np.asarray(labels)
    W_ih = np.asarray(W_ih, f32)
    We = W_ih[:, :E]                     # [G, E]
    Wx = W_ih[:, E:]                     # [G, H]

    Ball = cnn_feat.shape[0]
    emb = np.asarray(embed_table, f32)[labels]               # [128, 17, E]
    emb_in = np.concatenate(
        [np.broadcast_to(np.asarray(sos, f32), (Ball, 1, E)), emb],
        axis=1)[:, :T]
    geh = emb_in @ We.T + np.asarray(b_ih, f32) + np.asarray(b_hh, f32)

    wcT = np.ascontiguousarray(np.asarray(Wc, f32).T).reshape(KH, 128, H).astype(bf)
    bcpk = np.ascontiguousarray(
        (np.asarray(bc, f32) + np.asarray(bh, f32)).reshape(KH, 128).T)
    wxT = np.ascontiguousarray(Wx.T).reshape(KH, 128, G).astype(bf)
    whhT = np.ascontiguousarray(np.asarray(W_hh, f32).T).reshape(KH, 128, G).astype(bf)
    whT = np.ascontiguousarray(np.asarray(Wh, f32).T).reshape(KH, 128, H).astype(bf)
    wcls_pad = np.zeros((CT * 128, H), f32)
    wcls_pad[:C] = np.asarray(Wcls, f32)
    wclsT = np.ascontiguousarray(wcls_pad.T).reshape(KH, 128, CT * 128).astype(bf)
    vT8 = np.ascontiguousarray(
        np.asarray(v_w, f32).reshape(KH, 128).T).astype(bf)  # [128, KH]
    h0 = np.asarray(h0, f32)
    h0b = np.ascontiguousarray(np.broadcast_to(h0, (B, H)), f32)
    hpk0 = np.ascontiguousarray(np.broadcast_to(
        h0.reshape(KH, 128, 1), (KH, 128, B)).transpose(1, 0, 2).reshape(128, 128)).astype(bf)

    in_maps = []
    for core in range(NCORES):
        b0 = core * B
        fc = cnn_feat[b0:b0 + B]                     # [16, 196, 1024]
        featp = np.zeros((B, 256, H), f32)
        featp[:, :N, :] = fc
        featp = featp.reshape(KB, 128, H).astype(bf)
        # featTc: [NSC, 128, KH, SCW]; col = b*196 + n  (b-major)
        ftT = np.ascontiguousarray(
            fc.reshape(NB, H).T)                     # [H, NB]
        featTc = np.ascontiguousarray(
            ftT.reshape(KH, 128, NSC, SCW).transpose(2, 1, 0, 3)).astype(bf)
        gepack = np.ascontiguousarray(
            geh[b0:b0 + B].transpose(1, 0, 2)).astype(bf)    # [T, B, G]
        in_maps.append({
            "featTc": featTc,
            "featp": featp,
            "wcT": wcT,
            "bcpk": bcpk,
            "wxT": wxT,
            "whhT": whhT,
            "whT": whT,
            "wclsT": wclsT,
            "vT": vT8,
            "ge": gepack,
            "h0b": h0b,
            "hpk0": hpk0,
        })
    return in_maps


def kernel(cnn_feat, labels, lens, sos, h0, embed_table, W_ih, b_ih, W_hh,
           b_hh, Wh, bh, Wc, bc, v_w, v_b, Wcls, bcls):
    # v_b shifts all scores uniformly -> softmax-invariant -> dropped.
    nc = _get_program()
    in_maps = _pack_inputs(cnn_feat, labels, sos, h0, embed_table, W_ih, b_ih,
                           W_hh, b_hh, Wh, bh, Wc, bc, v_w, Wcls)
    res = run_bass_kernel_spmd(nc, in_maps, list(range(NCORES)))
    outs = []
    bcls = np.asarray(bcls, np.float32)
    for core in range(NCORES):
        o = np.asarray(res.results[core]["out"], np.float32)  # [CT,128,TB]
        o = o.reshape(CT * 128, T, B)                         # [1024, T, B]
        o = o[:C].transpose(2, 1, 0)                          # [B, T, C]
        outs.append(o)
    full = np.concatenate(outs, axis=0) + bcls                # [128, T, C]
    return np.ascontiguousarray(full, np.float32)


if __name__ == "__main__":
    rng = np.random.default_rng(0)
    s = 0.02
    inputs = dict(
        cnn_feat=rng.standard_normal((128, N, H), dtype=np.float32),
        labels=rng.integers(0, C, (128, 17)).astype(np.int32),
        lens=rng.integers(1, 17, (128,)).astype(np.int32),
        sos=(rng.standard_normal(E) * s).astype(np.float32),
        h0=(rng.standard_normal(H) * s).astype(np.float32),
        embed_table=(rng.standard_normal((C, E)) * s).astype(np.float32),
        W_ih=(rng.standard_normal((G, E + H)) * s).astype(np.float32),
        b_ih=np.zeros(G, np.float32),
        W_hh=(rng.standard_normal((G, H)) * s).astype(np.float32),
        b_hh=np.zeros(G, np.float32),
        Wh=(rng.standard_normal((H, H)) * s).astype(np.float32),
        bh=np.zeros(H, np.float32),
        Wc=(rng.standard_normal((H, H)) * s).astype(np.float32),
        bc=np.zeros(H, np.float32),
        v_w=(rng.standard_normal(H) * s).astype(np.float32),
        v_b=np.zeros((), np.float32),
        Wcls=(rng.standard_normal((C, H)) * s).astype(np.float32),
        bcls=np.zeros(C, np.float32),
    )
    out = kernel(**inputs)
    print("out", out.shape, out.dtype, float(np.abs(out).max()))
